# revision 1
# baseline (speedup 1.0000x reference)
"""Trainium2 kernel for nn_KeyedLayer: out = (W_sparse @ x.T).T

W is [16384, 16384] sparse COO (rows sorted, ~128 nnz/row, 2M nnz),
x is [64, 16384] fp32.  Strategy v1: shard output rows across 8 cores
(2048 rows each; disjoint outputs, no collectives).  Each core computes a
dense fp16 matmul  out_core[64, 2048] = x @ W_core.T  with W_core.T
densified on the host, K=16384 contracted in 128 blocks through PSUM.
"""

import os
from contextlib import ExitStack

import numpy as np
import ml_dtypes

import concourse.bass as bass
import concourse.tile as tile
from concourse import bacc, mybir
from concourse.bass_utils import run_bass_kernel_spmd

B = 64
IN_DIM = 16384
OUT_DIM = 16384
N_CORES = 8
ROWS_PER_CORE = OUT_DIM // N_CORES  # 2048
KBLK = IN_DIM // 128  # 128 k-blocks of 128
KGRP = int(os.environ.get("KERNEL_KGRP", "1"))  # k-blocks per DMA group
NGRP = KBLK // KGRP
WBUFS = int(os.environ.get("KERNEL_WBUFS", "16"))
NQ = ROWS_PER_CORE // 512  # 4 psum banks of 512 fp32

BF16 = mybir.dt.float16
F32 = mybir.dt.float32

_CACHE = {}

LAST_RESULT = None  # BassKernelResults of the most recent run (for test.py)


def _build_program():
    if "nc" in _CACHE:
        return _CACHE["nc"]
    nc = bacc.Bacc(
        "TRN2", target_bir_lowering=False, debug=False, num_devices=N_CORES
    )
    xT_d = nc.dram_tensor("xT", [128, KBLK, B], BF16, kind="ExternalInput")
    wt_d = nc.dram_tensor("wt", [NGRP, 128, KGRP * ROWS_PER_CORE], BF16,
                          kind="ExternalInput")
    out_d = nc.dram_tensor("out", [B, ROWS_PER_CORE], F32, kind="ExternalOutput")

    with tile.TileContext(nc) as tc, ExitStack() as ctx:
        xpool = ctx.enter_context(tc.tile_pool(name="x", bufs=1))
        wpool = ctx.enter_context(tc.tile_pool(name="w", bufs=WBUFS))
        opool = ctx.enter_context(tc.tile_pool(name="o", bufs=1))
        pspool = ctx.enter_context(
            tc.tile_pool(name="ps", bufs=1, space=bass.MemorySpace.PSUM)
        )

        xsb = xpool.tile([128, KBLK, B], BF16)  # 2 MiB
        nc.sync.dma_start(xsb[:], xT_d[:])

        psum = pspool.tile([B, NQ, 512], F32)  # [64, 2048] = 4 banks

        for g in range(NGRP):
            wsb = wpool.tile([128, KGRP, ROWS_PER_CORE], BF16)  # 2 MiB
            nc.sync.dma_start(wsb[:], wt_d[g])
            for j in range(KGRP):
                k = g * KGRP + j
                for q in range(NQ):
                    nc.tensor.matmul(
                        psum[:, q, :],
                        xsb[:, k, :],                    # lhsT [128, 64]
                        wsb[:, j, q * 512:(q + 1) * 512],  # rhs [128, 512]
                        start=(k == 0),
                        stop=(k == KBLK - 1),
                        skip_group_check=True,
                    )

        # Per-bank copyback so each bank's store DMA overlaps the next copy.
        osb = opool.tile([B, NQ, 512], F32)
        for q in range(NQ):
            nc.vector.tensor_copy(osb[:, q, :], psum[:, q, :])
            nc.sync.dma_start(
                out_d.ap().rearrange("b (q n) -> b q n", q=NQ)[:, q, :],
                osb[:, q, :],
            )

    nc.compile()
    _CACHE["nc"] = nc
    return nc


def kernel(x_affine: np.ndarray, rows: np.ndarray, cols: np.ndarray,
           vals: np.ndarray) -> np.ndarray:
    global LAST_RESULT
    import scipy.sparse as sp

    x_affine = np.asarray(x_affine, dtype=np.float32)
    rows = np.asarray(rows, dtype=np.int64)
    cols = np.asarray(cols, dtype=np.int64)
    vals = np.asarray(vals, dtype=np.float32)

    # xT host layout [p, k, b]: element = x[b, k*128 + p]
    xT = np.ascontiguousarray(
        x_affine.T.reshape(KBLK, 128, B).transpose(1, 0, 2)
    ).astype(np.float16)

    # rows is sorted; slice each core's nnz range and densify only its
    # [16384, 2048] W.T block (duplicates are summed by scipy).
    in_maps = []
    for c in range(N_CORES):
        base = c * ROWS_PER_CORE
        m = (rows >= base) & (rows < base + ROWS_PER_CORE)
        w_slice = sp.coo_matrix(
            (vals[m], (cols[m], rows[m] - base)),
            shape=(IN_DIM, ROWS_PER_CORE),
        ).toarray()  # [16384, 2048] fp32
        # [g, p, j, n] with in-dim = (g*KGRP + j)*128 + p
        wt = np.ascontiguousarray(
            w_slice.reshape(NGRP, KGRP, 128, ROWS_PER_CORE).transpose(0, 2, 1, 3)
        ).astype(np.float16).reshape(NGRP, 128, KGRP * ROWS_PER_CORE)
        in_maps.append({"xT": xT, "wt": wt})

    nc = _build_program()
    res = run_bass_kernel_spmd(
        nc, in_maps, list(range(N_CORES)),
        trace=bool(int(os.environ.get("KERNEL_TRACE", "0"))),
    )
    LAST_RESULT = res
    out = np.concatenate(
        [res.results[i]["out"] for i in range(N_CORES)], axis=1
    )
    return out.astype(np.float32)

